# revision 31
# baseline (speedup 1.0000x reference)
"""Trainium2 Bass kernel for ContinuousWaveletLayer (CWT energy).

Reference computation:
  bank = Morlet wavelet bank [32 scales, Lmax=256] (static)
  coef[b,s,t] = 'same' conv of x[b,:] (len 8192) with bank[s,:]
  out[b,s]    = mean_t(coef^2) * softmax(scale_weights)[s]

Device strategy (8 NeuronCores, scale-parallel, 4 scales/core), v2:
  Batch-on-partitions layout.  With x zero-padded (128/128) and viewed
  time-major in blocks of 128, the output block B (128 time steps, all
  4 scales) is computed as

      coef_B[b, (s,to)] = sum_{D=0..2} Xblk[B+D].T @ G[D]      (K=128)

  where Xblk[A][a,b] = xpad[128*A + a, b] is the STATIONARY operand
  (loaded once per A in an A-major loop; each block's accumulation
  group spans A=B..B+2 in its own PSUM bank) and G[D][a,(s,to)] =
  g_s[128*D + a - to] is the 512-wide moving operand shared by all
  blocks.  PSUM tiles [b=128, (s=4, to=128)] are squared+cast to bf16
  on ACT (1 of 8 blocks on DVE to balance), then a single DVE
  tensor_reduce(axis=X) per block folds `to`, appending [128, 4]
  per-block partial energies to an SBUF [128, 64*4] buffer — the PE
  never touches the reduction.  Host folds blocks, applies 1/N and
  softmax (O(4K) flops).
"""

import os
import sys
from contextlib import ExitStack

import numpy as np

sys.path.insert(0, "/opt/trn_rl_repo")

import concourse.bass as bass
import concourse.mybir as mybir
from concourse import tile
from concourse.bass_utils import run_bass_kernel_spmd
from concourse.vector_clock import ScopedClock


def _drain_and_barrier_single_wait(self, tick_clock, wait_clock):
    """TileContext._drain_and_barrier, but the kernel-tail drain's
    global-clock waits are spread over a chain of single-wait drains —
    the walrus build here allows only one sync wait per instruction."""
    drain_inst = self.nc.sync.drain()
    wait_clock.add_sem_waits(
        drain_inst.ins, ScopedClock({None: tick_clock.global_clock})
    )
    si = drain_inst.ins.sync_info
    waits = list(si.on_wait)
    if len(waits) > 1:
        si.on_wait = [waits[0]]
        sems = {h.name: h for h in self.sems.allocated().values()}
        for w in waits[1:]:
            d2 = self.nc.sync.drain()
            d2.wait_op(sems[w.ant_name], w.wait_value, "sem-ge")
    self.nc.all_engine_barrier()
    assert self.sems is not None
    popped = self.nc._tile_sem_poison_stack.pop()
    assert popped is self._sem_poison
    self.nc.clear_and_free_semaphores(list(self.sems.allocated().values()))
    self.nc.all_engine_barrier()


tile.TileContext._drain_and_barrier = _drain_and_barrier_single_wait

N_CORES = 8
S_TOTAL = 32          # number of scales
S_PER = 4             # scales per core
P = 128               # partition / block size
NT = 8192             # time samples
LMAX = 256            # padded kernel length
NBLK = 66             # input blocks: (128 + 8192 + 128) / 128
NOUT = 64             # output blocks: 8192 / 128
F32 = mybir.dt.float32
BF16 = mybir.dt.bfloat16

GCOLS = 3 * S_PER * P          # 1536 moving-operand (G) columns
XCOLS = NBLK * P               # 8448 x columns

# x chunk boundaries (in blocks) for the pipelined input DMA; chunk 0 is
# small so the conv starts early, later chunks hide behind compute
XCHUNKS = [(0, 4), (4, 16), (16, 40), (40, 66)]

# blocks evicted via DVE-copy + GPSIMD-square instead of ACT-square
# (empty: the GPSIMD offload measured slower — it serializes the tail)
DVE_EVICT = frozenset()
# from this block on, GPSIMD pair-sums sq tiles and DVE reduces per pair
PAIR0 = NOUT

LAST_RESULTS = None   # BassKernelResults of the most recent run (for test.py)


def _morlet_kernel_bank(n_scales: int, n: int) -> np.ndarray:
    Lmax = min(8 * n_scales, n)
    bank = np.zeros((n_scales, Lmax), dtype=np.float32)
    for i, s in enumerate(range(1, n_scales + 1)):
        L = min(8 * s, n)
        t = np.linspace(-4.0 * s, 4.0 * s, L)
        w = np.exp(-t**2 / (2.0 * s**2)) * np.cos(5.0 * t / s)
        w = w / np.sqrt(s)
        off = (Lmax - 1) // 2 - (L - 1) // 2
        bank[i, off : off + L] = w.astype(np.float32)
    return bank


def _toeplitz_weights() -> np.ndarray:
    """G[s, D][a, to] = g_s[128*D + a - to], zero outside support."""
    bank = _morlet_kernel_bank(S_TOTAL, NT)          # [32, 256]
    g = bank[:, ::-1].copy()                         # reversed rows
    a = np.arange(P)[:, None]
    to = np.arange(P)[None, :]
    G = np.zeros((S_TOTAL, 3, P, P), dtype=np.float32)
    for D in range(3):
        d = 128 * D + a - to
        valid = (d >= 0) & (d < LMAX)
        dc = np.clip(d, 0, LMAX - 1)
        for s in range(S_TOTAL):
            G[s, D] = np.where(valid, g[s][dc], 0.0)
    return G


def _build_nc() -> bass.Bass:
    nc = bass.Bass()
    # single input tensor: [ G bank (D,s,to) | x time-major (A,b) ], bf16
    xg = nc.dram_tensor("xg", [P, GCOLS + XCOLS], BF16, kind="ExternalInput")
    # per-core per-block partial energies: [b, (B, s)]
    outp = nc.dram_tensor("outp", [P, NOUT * S_PER], BF16, kind="ExternalOutput")

    with tile.TileContext(nc) as tc, ExitStack() as ctx:
        # few pools: every pool pays all-engine-barrier rounds at teardown
        inpool = ctx.enter_context(tc.tile_pool(name="in", bufs=1))
        # one sq buffer per block: no slot reuse → no WAR-induced second
        # wait on the squares (walrus allows 1 sync wait/instruction)
        sqpool = ctx.enter_context(tc.tile_pool(name="sq", bufs=NOUT))
        pspool = ctx.enter_context(tc.tile_pool(name="ps", bufs=7, space="PSUM"))
        jpool = ctx.enter_context(tc.tile_pool(name="j", bufs=1, space="PSUM"))

        gsb = inpool.tile([P, GCOLS], BF16, tag="g", name="g")
        xts = [
            inpool.tile(
                [P, (b1 - b0) * P], BF16, tag=f"xt{i}", name=f"xt{i}"
            )
            for i, (b0, b1) in enumerate(XCHUNKS)
        ]
        # pre-warm the PE while the input DMA is in flight: ~3.5us of junk
        # matmuls lift the HAM clock gate to 8/8 before real data lands
        wtile = inpool.tile([P, P], BF16, tag="wt", name="wt")
        jp = jpool.tile([P, P], F32, tag="jp", name="jp")
        nc.gpsimd.memset(wtile[:, :], 0.0)
        for _ in range(32):
            nc.tensor.matmul(jp[:, :], wtile[:, :], wtile[:, :], start=True, stop=True)

        # pipelined input DMA, all on the sync queue IN ORDER: the SDMA
        # rings process descriptors in arrival order, so G + chunk 0
        # complete first and the conv starts after ~0.5 MB, not 2.5 MB
        nc.sync.dma_start(out=gsb[:, :], in_=xg[:, :GCOLS])
        for i, (b0, b1) in enumerate(XCHUNKS):
            nc.sync.dma_start(
                out=xts[i][:, :], in_=xg[:, GCOLS + b0 * P : GCOLS + b1 * P]
            )

        def xblk(A: int):
            for i, (b0, b1) in enumerate(XCHUNKS):
                if b0 <= A < b1:
                    return xts[i][:, (A - b0) * P : (A - b0 + 1) * P]
            raise AssertionError(A)

        esum = inpool.tile([P, NOUT * S_PER], BF16, tag="esum", name="esum")

        # A-major conv loop: stationary Xblk[A] serves D=2,1,0 (descending
        # so a chunk-boundary DMA wait lands on a mid-group matmul, never
        # on a start=True matmul that already carries a PSUM WAR wait)
        pts = {}
        sqs = {}
        for A in range(NBLK):
            for D in (2, 1, 0):
                B = A - D
                if not (0 <= B < NOUT):
                    continue
                if D == 0:
                    # first matmul of block B's accumulation group
                    pts[B] = pspool.tile([P, 512], F32, tag="pt", name="pt")
                nc.tensor.matmul(
                    pts[B][:, :],
                    xblk(A),
                    gsb[:, D * 512 : (D + 1) * 512],
                    start=(D == 0),
                    stop=(D == 2),
                )
            B = A - 2
            if 0 <= B < NOUT:
                pt = pts.pop(B)
                sq = sqpool.tile([P, 512], BF16, tag="sq", name="sq")
                if B in DVE_EVICT:
                    # relieve ACT: DVE casts the psum tile to bf16, the idle
                    # GPSIMD squares it (engines stay at 1 sync wait each)
                    cast = sqpool.tile([P, 512], BF16, tag="sq", name="sq")
                    nc.vector.tensor_copy(cast[:, :], pt[:, :])
                    nc.gpsimd.tensor_mul(sq[:, :], cast[:, :], cast[:, :])
                else:
                    nc.scalar.square(sq[:, :], pt[:, :])
                sqs[B] = sq
                if B < PAIR0:
                    # unpaired block: one DVE reduce per block
                    with nc.allow_low_precision(
                        reason="bf16 per-block partials; host folds in fp32"
                    ):
                        nc.vector.tensor_reduce(
                            out=esum[:, B * S_PER : (B + 1) * S_PER],
                            in_=sq[:, :].rearrange("p (s t) -> p s t", s=S_PER),
                            axis=mybir.AxisListType.X,
                            op=mybir.AluOpType.add,
                        )
                elif B % 2 == 1:
                    # paired region: GPSIMD sums the pair of sq tiles, DVE
                    # reduces once per pair into the even block's slot
                    ss = sspool.tile([P, 512], BF16, tag="ss", name="ss")
                    nc.gpsimd.tensor_add(ss[:, :], sqs[B - 1][:, :], sq[:, :])
                    with nc.allow_low_precision(
                        reason="bf16 per-pair partials; host folds in fp32"
                    ):
                        nc.vector.tensor_reduce(
                            out=esum[:, (B - 1) * S_PER : B * S_PER],
                            in_=ss[:, :].rearrange("p (s t) -> p s t", s=S_PER),
                            axis=mybir.AxisListType.X,
                            op=mybir.AluOpType.add,
                        )
        assert not pts
        nc.sync.dma_start(out=outp[:, :], in_=esum[:, :])

    return nc


_NC_CACHE = None


def _get_nc() -> bass.Bass:
    global _NC_CACHE
    if _NC_CACHE is None:
        _NC_CACHE = _build_nc()
    return _NC_CACHE


def _ensure_ntff_hook() -> None:
    """Install an `antenv.axon_hooks` shim (missing on this image) so
    run_bass_kernel_spmd(trace=True) can drive NTFF profiling through
    libaxon_pjrt.so's C ABI. Trace-path only; harness runs never hit it."""
    import contextlib
    import ctypes
    import types

    if "antenv.axon_hooks" in sys.modules:
        return
    so_path = os.environ.get("PJRT_LIBRARY_PATH", "/opt/axon/libaxon_pjrt.so")
    try:
        lib = ctypes.CDLL(so_path)
        lib.axon_start_nrt_profile.argtypes = [
            ctypes.POINTER(ctypes.c_int64),
            ctypes.c_size_t,
        ]
        lib.axon_start_nrt_profile.restype = ctypes.c_int64
        lib.axon_stop_nrt_profile.argtypes = [ctypes.c_char_p]
        lib.axon_stop_nrt_profile.restype = ctypes.c_int64
    except (OSError, AttributeError):
        return

    @contextlib.contextmanager
    def _hook(output_dir, device_ids):
        import jax

        jax.devices()
        if device_ids:
            ids = (ctypes.c_int64 * len(device_ids))(*device_ids)
            rc = lib.axon_start_nrt_profile(ids, len(device_ids))
        else:
            rc = lib.axon_start_nrt_profile(None, 0)
        if rc != 0:
            raise RuntimeError(f"axon_start_nrt_profile rc={rc}")
        try:
            yield
        finally:
            n = lib.axon_stop_nrt_profile(str(output_dir).encode())
            if n < 0:
                raise RuntimeError(f"axon_stop_nrt_profile rc={n}")
            if n == 0:
                print(f"profile: ZERO files written to {output_dir}", file=sys.stderr)

    mod = types.ModuleType("antenv.axon_hooks")
    mod.get_axon_ntff_profile_hook = lambda: _hook
    mod.set_axon_ntff_profile_hook = lambda h: None
    import antenv

    sys.modules["antenv.axon_hooks"] = mod
    antenv.axon_hooks = mod


def kernel(x: np.ndarray, scale_weights: np.ndarray, _trace: bool = False) -> np.ndarray:
    global LAST_RESULTS
    x = np.asarray(x, dtype=np.float32)
    scale_weights = np.asarray(scale_weights, dtype=np.float32)
    assert x.shape == (P, NT) and scale_weights.shape == (S_TOTAL,)

    # host prep: zero-pad, transpose to time-major blocked layout
    xpad = np.zeros((NBLK * P, P), dtype=np.float32)
    xpad[P : P + NT, :] = x.T
    # xb2[a, A*128 + b] = xpad[A*128 + a, b]
    xb2 = np.ascontiguousarray(
        xpad.reshape(NBLK, P, P).transpose(1, 0, 2).reshape(P, NBLK * P)
    )

    G = _toeplitz_weights()  # [32, 3, 128, 128]
    import ml_dtypes

    bf16 = ml_dtypes.bfloat16
    xgs = []
    for c in range(N_CORES):
        Gc = G[c * S_PER : (c + 1) * S_PER]            # [s, D, a, to]
        Gb = Gc.transpose(2, 1, 0, 3).reshape(P, GCOLS)  # [a, (D, s, to)]
        xgs.append(
            np.ascontiguousarray(np.concatenate([Gb, xb2], axis=1).astype(bf16))
        )

    nc = _get_nc()
    in_maps = [{"xg": xgs[c]} for c in range(N_CORES)]
    if _trace:
        _ensure_ntff_hook()
    res = run_bass_kernel_spmd(nc, in_maps, list(range(N_CORES)), trace=_trace)
    LAST_RESULTS = res

    # gather + unshard: [8 cores][128, (B=64, s=4)] -> [128, 32]; only
    # slots [0,32) and even slots >= 32 hold partials (pairs fold 2 blocks)
    esum = np.stack(
        [res.results[c]["outp"].astype(np.float32).reshape(P, NOUT, S_PER) for c in range(N_CORES)],
        axis=0,
    )  # [8, 128, 64, 4]
    valid = np.r_[np.arange(PAIR0), np.arange(PAIR0, NOUT, 2)]
    energy = esum[:, :, valid, :].sum(axis=2).transpose(1, 0, 2).reshape(P, S_TOTAL) / np.float32(NT)

    w = scale_weights.astype(np.float64)
    e = np.exp(w - w.max())
    sm = (e / e.sum()).astype(np.float32)
    return (energy * sm[None, :]).astype(np.float32)


if __name__ == "__main__":
    rng = np.random.default_rng(0)
    x = rng.standard_normal((P, NT), dtype=np.float32)
    sw = rng.standard_normal(S_TOTAL, dtype=np.float32)
    out = kernel(x, sw)
    print("kernel output shape:", out.shape, out.dtype)
